# revision 13
# baseline (speedup 1.0000x reference)
"""SNN ActorCritic TRN2 kernel.

Math (per layer, snntorch Leaky, reset_mechanism='subtract', thresh=1):
    d_t = 0.95*d_{t-1} + (cur_t - 0.05) - s_{t-1},  s_t = (d_t > 0),  d = mem - 1

Rescaled state (kills the 0.95 multiply):  D_t = d_t / 0.95^t
    D_t = D_{t-1} + kappa_t * (g_t + beta - s_{t-1}),  kappa_t = 0.95^-t
where g_t = x_t @ W.T (matmul, no bias), beta = b - 0.05.
Kernel accumulates D'_t = sum kappa_tau*(g_tau - s_{tau-1}); the
deterministic beta part is folded into per-step thresholds:
    s_t = (D'_t > theta_t),  theta_t = LAM - beta*S_t  (LAM from d_{-1} = -1)

Per step on device:
    psum1[h,b] = sum_j W1hi/lo @ x_t  (2-term fp32r split, exact for binary x)
                 + (-I) @ s_{t-1}     (spike subtract via PE)
    D' += kappa_t * psum1             (DVE scalar_tensor_tensor, psum src)
    s   = (D' > theta_t)              (DVE tensor_scalar per chunk, fp32r out)
    psum2 = W2hi/lo @ s + (-I21) @ s2
    D2' += kappa_t * psum2;  s2 = (D2' > theta2_t)
    acc_psum += G @ s2                (spike counts, accumulated all T steps)

Outputs per core: acc [2,Bc] (policy logits = spike counts), D2'[20].
Host: softmax + affine reconstruction of m_c2.
"""
import numpy as np
import concourse.bass as bass
from concourse import bacc
import concourse.mybir as mybir
import concourse.tile as tile
from concourse.bass_utils import run_bass_kernel_spmd

dt = mybir.dt
ALU = mybir.AluOpType
AF = mybir.ActivationFunctionType

LAM = 0.95
D_IN = 128
H = 256
HT = 512          # actor H + critic H
NCH = 4           # HT / 128 partition chunks
NPA = 10
NOUT = 21         # 20 actor + 1 critic
N_CORES = 8


def split_fp32r(a):
    """Split fp32 array into hi+lo, both valid fp32r (low 12 mantissa bits 0),
    with hi+lo == a exactly."""
    a = np.ascontiguousarray(a, dtype=np.float32)
    hi = (a.view(np.uint32) & np.uint32(0xFFFFF000)).view(np.float32)
    lo = (a - hi).astype(np.float32)
    assert (lo.view(np.uint32) & np.uint32(0xFFF) == 0).all(), "lo not fp32r-valid"
    assert ((hi + lo) == a).all(), "split not exact"
    return hi, lo


def kappas_f32(T):
    return np.float64(LAM) ** (-np.arange(T, dtype=np.float64))


def build_nc(T, Bc):
    """Build the Bass module for T steps, per-core batch Bc."""
    f32, f32r = dt.float32, dt.float32r
    nc = bacc.Bacc("TRN2", target_bir_lowering=False)

    xT_d = nc.dram_tensor("xT", [T, D_IN, Bc], f32r, kind="ExternalInput")
    w1hi_d = nc.dram_tensor("w1hi", [D_IN, HT], f32r, kind="ExternalInput")
    w1lo_d = nc.dram_tensor("w1lo", [D_IN, HT], f32r, kind="ExternalInput")
    w2hi_d = nc.dram_tensor("w2hi", [128, NCH * NOUT], f32r, kind="ExternalInput")
    w2lo_d = nc.dram_tensor("w2lo", [128, NCH * NOUT], f32r, kind="ExternalInput")
    negI_d = nc.dram_tensor("negI", [128, 128], f32r, kind="ExternalInput")
    negI21_d = nc.dram_tensor("negI21", [NOUT, NOUT], f32r, kind="ExternalInput")
    g_d = nc.dram_tensor("gmat", [NOUT, 2], f32r, kind="ExternalInput")
    th1_d = nc.dram_tensor("th1", [128, NCH * T], f32, kind="ExternalInput")
    th2_d = nc.dram_tensor("th2", [NOUT, T], f32, kind="ExternalInput")
    acc_d = nc.dram_tensor("acc", [2, Bc], f32, kind="ExternalOutput")
    d2c_d = nc.dram_tensor("d2c", [1, Bc], f32, kind="ExternalOutput")

    kap = [float(np.float32(k)) for k in kappas_f32(T)]

    FB = NCH * Bc                       # free width of packed layer-1 state
    REG = 2048 // 4                     # psum zero-region width in fp32 elems
    reg_w = min(FB, REG)
    n_reg = FB // reg_w
    ch_per_reg = reg_w // Bc

    with tile.TileContext(nc) as tc:
        with tc.tile_pool(name="consts", bufs=1) as consts, \
             tc.tile_pool(name="state", bufs=1) as state, \
             tc.tile_pool(name="xp", bufs=8) as xp, \
             tc.tile_pool(name="ps1p", bufs=2, space="PSUM") as ps1p, \
             tc.tile_pool(name="ps2p", bufs=2, space="PSUM") as ps2p, \
             tc.tile_pool(name="accp", bufs=1, space="PSUM") as accp:

            w1hi = consts.tile([D_IN, HT], f32r)
            w1lo = consts.tile([D_IN, HT], f32r)
            w2hi = consts.tile([128, NCH * NOUT], f32r)
            w2lo = consts.tile([128, NCH * NOUT], f32r)
            negI = consts.tile([128, 128], f32r)
            negI21 = consts.tile([NOUT, NOUT], f32r)
            gmat = consts.tile([NOUT, 2], f32r)
            th1 = consts.tile([128, NCH * T], f32)
            th2 = consts.tile([NOUT, T], f32)
            for tt, dd in ((w1hi, w1hi_d), (w1lo, w1lo_d), (w2hi, w2hi_d),
                           (w2lo, w2lo_d), (negI, negI_d), (negI21, negI21_d),
                           (gmat, g_d), (th1, th1_d), (th2, th2_d)):
                nc.gpsimd.dma_start(out=tt, in_=dd[:, :])

            Dp = state.tile([128, FB], f32)      # layer-1 D' (4 chunks packed)
            s1 = state.tile([128, FB], f32r)     # layer-1 spikes {0,1}
            D2 = state.tile([NOUT, Bc], f32)
            s2 = state.tile([NOUT, Bc], f32r)
            accsb = state.tile([2, Bc], f32)
            nc.vector.memset(Dp, 0.0)
            nc.vector.memset(D2, 0.0)
            # fp32r tiles can't be memset directly; produce rounded zeros via
            # an always-false compare (also satisfies the fp32r producer rule)
            nc.vector.tensor_scalar(out=s1, in0=Dp, scalar1=1e30, scalar2=None,
                                    op0=ALU.is_gt)
            nc.vector.tensor_scalar(out=s2, in0=D2, scalar1=1e30, scalar2=None,
                                    op0=ALU.is_gt)

            # Priming matmuls: each PE-consumed const gets one dummy matmul so
            # every later matmul carries at most one semaphore wait
            # (walrus rejects fp32r matmuls with >1 sync wait).
            pdum = ps2p.tile([NOUT, Bc], f32, tag="ps2")
            for cst, kk, mm in ((w1hi, D_IN, NOUT), (w1lo, D_IN, NOUT),
                                (w2hi, 128, NOUT), (w2lo, 128, NOUT),
                                (negI, 128, NOUT), (negI21, NOUT, NOUT),
                                (gmat, NOUT, 2)):
                nc.tensor.matmul(pdum[:mm, :2], cst[:kk, :mm], cst[:kk, :2],
                                 start=True, stop=True)

            acc_ps = accp.tile([2, Bc], f32)

            for t in range(T):
                x_t = xp.tile([D_IN, Bc], f32r)
                # alternate HWDGE rings (SP / ACT) for the x loads
                (nc.sync if t % 2 == 0 else nc.scalar).dma_start(
                    out=x_t, in_=xT_d[t, :, :])

                ps1 = ps1p.tile([128, FB], f32)
                for r in range(n_reg):
                    rsl = slice(r * reg_w, (r + 1) * reg_w)
                    # feed-forward matmuls first (off the recurrence chain)
                    for jj in range(ch_per_reg):
                        j = r * ch_per_reg + jj
                        sl = slice(j * Bc, (j + 1) * Bc)
                        wsl = slice(j * 128, (j + 1) * 128)
                        nc.tensor.matmul(ps1[:, sl], w1hi[:, wsl], x_t,
                                         start=(jj == 0), stop=False)
                        nc.tensor.matmul(ps1[:, sl], w1lo[:, wsl], x_t,
                                         start=False, stop=False)
                    # spike subtract for the whole region; last in group
                    nc.tensor.matmul(ps1[:, rsl], negI, s1[:, rsl],
                                     start=False, stop=True)
                # D' += kappa_t * psum  (one packed op, fewer DVE drains)
                nc.vector.scalar_tensor_tensor(
                    out=Dp, in0=ps1, scalar=kap[t], in1=Dp,
                    op0=ALU.mult, op1=ALU.add)
                # s = (D' > theta_t) per chunk
                for j in range(NCH):
                    sl = slice(j * Bc, (j + 1) * Bc)
                    nc.vector.tensor_scalar(
                        out=s1[:, sl], in0=Dp[:, sl],
                        scalar1=th1[:, j * T + t: j * T + t + 1],
                        scalar2=None, op0=ALU.is_gt)

                # layer 2
                ps2 = ps2p.tile([NOUT, Bc], f32)
                for j in range(NCH):
                    sl = slice(j * Bc, (j + 1) * Bc)
                    wsl = slice(j * NOUT, (j + 1) * NOUT)
                    nc.tensor.matmul(ps2, w2hi[:, wsl], s1[:, sl],
                                     start=(j == 0), stop=False)
                    nc.tensor.matmul(ps2, w2lo[:, wsl], s1[:, sl],
                                     start=False, stop=False)
                nc.tensor.matmul(ps2, negI21, s2, start=False, stop=True)

                nc.vector.scalar_tensor_tensor(
                    out=D2, in0=ps2, scalar=kap[t], in1=D2,
                    op0=ALU.mult, op1=ALU.add)
                nc.vector.tensor_scalar(
                    out=s2, in0=D2, scalar1=th2[:, t:t + 1], scalar2=None,
                    op0=ALU.is_gt)

                # spike counting: acc += G @ s2
                nc.tensor.matmul(acc_ps, gmat, s2,
                                 start=(t == 0), stop=(t == T - 1))

            nc.vector.tensor_copy(accsb, acc_ps)
            nc.sync.dma_start(out=acc_d[:, :], in_=accsb)
            nc.sync.dma_start(out=d2c_d[:, :], in_=D2[NOUT - 1:NOUT, :])

    nc.compile()
    return nc


def host_prep(T, Bc, W1a, b1a, W2a, b2a, W1c, b1c, W2c, b2c):
    """Precompute all constant arrays shared by every core."""
    W1 = np.vstack([W1a, W1c]).astype(np.float32)          # [512, 128]
    w1hi, w1lo = split_fp32r(np.ascontiguousarray(W1.T))   # [128, 512]

    W2blk = np.zeros((NOUT, HT), np.float32)
    W2blk[:2 * NPA, :H] = W2a
    W2blk[2 * NPA:, H:] = W2c
    W2T = np.ascontiguousarray(W2blk.T)                    # [512, 21]
    w2t = np.concatenate([W2T[j * 128:(j + 1) * 128, :] for j in range(NCH)],
                         axis=1)                           # [128, 84]
    w2hi, w2lo = split_fp32r(w2t)

    negI = np.ascontiguousarray(-np.eye(128, dtype=np.float32))
    negI21 = np.ascontiguousarray(-np.eye(NOUT, dtype=np.float32))
    G = np.zeros((NOUT, 2), np.float32)
    G[:NPA, 0] = 1.0
    G[NPA:2 * NPA, 1] = 1.0

    kap32 = np.float32(kappas_f32(T))
    S = np.cumsum(kap32.astype(np.float64))                # S_t

    b1 = np.concatenate([b1a, b1c]).astype(np.float32)     # [512]
    b2 = np.concatenate([b2a, b2c]).astype(np.float32)     # [21]
    beta1 = b1.astype(np.float64) - 0.05
    beta2 = b2.astype(np.float64) - 0.05
    # d = mem - 1 starts at -1: decayed initial condition adds +LAM
    th1_full = (LAM - beta1[:, None] * S[None, :]).astype(np.float32)  # [512, T]
    th1 = np.ascontiguousarray(
        th1_full.reshape(NCH, 128, T).transpose(1, 0, 2).reshape(128, NCH * T))
    th2 = (LAM - beta2[:, None] * S[None, :]).astype(np.float32)       # [21, T]

    consts = dict(w1hi=w1hi, w1lo=w1lo, w2hi=w2hi, w2lo=w2lo, negI=negI,
                  negI21=negI21, gmat=G, th1=th1,
                  th2=np.ascontiguousarray(th2))
    return consts, S


def host_post(acc_list, d2c_list, S, T, b2c):
    """Per-core [2,Bc]/[1,Bc] lists -> (policy [B,2], m_c2 [B,1])."""
    acc = np.concatenate(acc_list, axis=1)                 # [2, B]
    d2c = np.concatenate(d2c_list, axis=1)[0]              # [B]
    av = acc.T.astype(np.float32)                          # [B, 2] action values
    m = av.max(axis=1, keepdims=True)
    e = np.exp(av - m, dtype=np.float32)
    policy = (e / e.sum(axis=1, keepdims=True)).astype(np.float32)

    lam99 = np.float64(LAM) ** (T - 1)
    beta2c = np.float64(b2c[0]) - 0.05
    m_c2 = (lam99 * (d2c.astype(np.float64) + beta2c * S[T - 1] - LAM) + 1.0)
    return policy, m_c2.astype(np.float32)[:, None]


def run_full(spikes, weights, T=None, n_cores=N_CORES, **spmd_kwargs):
    """spikes [T,B,D] fp32 binary; weights dict W1a..b2c. Returns
    (policy [B,2], m_c2 [B,1], BassKernelResults)."""
    T = T if T is not None else spikes.shape[0]
    B = spikes.shape[1]
    Bc = B // n_cores
    consts, S = host_prep(T, Bc, **weights)
    nc = build_nc(T, Bc)
    xT = np.ascontiguousarray(spikes.transpose(0, 2, 1))   # [T, 128, B]
    in_maps = []
    for c in range(n_cores):
        m = dict(consts)
        m["xT"] = np.ascontiguousarray(xT[:, :, c * Bc:(c + 1) * Bc])
        in_maps.append(m)
    res = run_bass_kernel_spmd(nc, in_maps, core_ids=list(range(n_cores)),
                               **spmd_kwargs)
    acc_list = [res.results[c]["acc"] for c in range(n_cores)]
    d2c_list = [res.results[c]["d2c"] for c in range(n_cores)]
    policy, m_c2 = host_post(acc_list, d2c_list, S, T, weights["b2c"])
    return policy, m_c2, res


# ----------------------------------------------------------------------------
# Harness entry point: kernel(**inputs) -> (policy [B,2], m_c2 [B,1])
# ----------------------------------------------------------------------------
_NC_CACHE = {}
LAST_EXEC_NS = None


def kernel(spikes, W1a, b1a, W2a, b2a, W1c, b1c, W2c, b2c):
    global LAST_EXEC_NS
    spikes = np.ascontiguousarray(np.asarray(spikes, dtype=np.float32))
    weights = dict(W1a=np.asarray(W1a), b1a=np.asarray(b1a),
                   W2a=np.asarray(W2a), b2a=np.asarray(b2a),
                   W1c=np.asarray(W1c), b1c=np.asarray(b1c),
                   W2c=np.asarray(W2c), b2c=np.asarray(b2c))
    T, B, _ = spikes.shape
    Bc = B // N_CORES
    key = (T, Bc)
    if key not in _NC_CACHE:
        _NC_CACHE[key] = build_nc(T, Bc)
    nc = _NC_CACHE[key]

    consts, S = host_prep(T, Bc, **weights)
    xT = np.ascontiguousarray(spikes.transpose(0, 2, 1))   # [T, 128, B]
    in_maps = []
    for c in range(N_CORES):
        m = dict(consts)
        m["xT"] = np.ascontiguousarray(xT[:, :, c * Bc:(c + 1) * Bc])
        in_maps.append(m)
    res = run_bass_kernel_spmd(nc, in_maps, core_ids=list(range(N_CORES)))
    LAST_EXEC_NS = getattr(res, "exec_time_ns", None)
    acc_list = [res.results[c]["acc"] for c in range(N_CORES)]
    d2c_list = [res.results[c]["d2c"] for c in range(N_CORES)]
    policy, m_c2 = host_post(acc_list, d2c_list, S, T, weights["b2c"])
    return policy, m_c2


# revision 17
# speedup vs baseline: 1.1871x; 1.1871x over previous
"""SNN ActorCritic TRN2 kernel.

Math (per layer, snntorch Leaky, reset_mechanism='subtract', thresh=1):
    d_t = 0.95*d_{t-1} + (cur_t - 0.05) - s_{t-1},  s_t = (d_t > 0),  d = mem - 1

Rescaled state (kills the 0.95 multiply):  D_t = d_t / 0.95^t
    D_t = D_{t-1} + kappa_t * (g_t + beta - s_{t-1}),  kappa_t = 0.95^-t
where g_t = x_t @ W.T (matmul, no bias), beta = b - 0.05.
Kernel accumulates D'_t = sum kappa_tau*(g_tau - s_{tau-1}); the
deterministic beta part is folded into per-step thresholds:
    s_t = (D'_t > theta_t),  theta_t = LAM - beta*S_t  (LAM from d_{-1} = -1)

Per step on device:
    psum1[h,b] = sum_j W1hi/lo @ x_t  (2-term fp32r split, exact for binary x)
                 + (-I) @ s_{t-1}     (spike subtract via PE)
    D' += kappa_t * psum1             (DVE scalar_tensor_tensor, psum src)
    s   = (D' > theta_t)              (DVE tensor_scalar per chunk, fp32r out)
    psum2 = W2hi/lo @ s + (-I21) @ s2
    D2' += kappa_t * psum2;  s2 = (D2' > theta2_t)
    acc_psum += G @ s2                (spike counts, accumulated all T steps)

Outputs per core: acc [2,Bc] (policy logits = spike counts), D2'[20].
Host: softmax + affine reconstruction of m_c2.
"""
import numpy as np
import concourse.bass as bass
from concourse import bacc
import concourse.mybir as mybir
import concourse.tile as tile
from concourse.bass_utils import run_bass_kernel_spmd

dt = mybir.dt
ALU = mybir.AluOpType
AF = mybir.ActivationFunctionType

LAM = 0.95
D_IN = 128
H = 256
HT = 512          # actor H + critic H
NCH = 4           # HT / 128 partition chunks
NPA = 10
NOUT = 21         # 20 actor + 1 critic
N_CORES = 8


def split_fp32r(a):
    """Split fp32 array into hi+lo, both valid fp32r (low 12 mantissa bits 0),
    with hi+lo == a exactly."""
    a = np.ascontiguousarray(a, dtype=np.float32)
    hi = (a.view(np.uint32) & np.uint32(0xFFFFF000)).view(np.float32)
    lo = (a - hi).astype(np.float32)
    assert (lo.view(np.uint32) & np.uint32(0xFFF) == 0).all(), "lo not fp32r-valid"
    assert ((hi + lo) == a).all(), "split not exact"
    return hi, lo


def kappas_f32(T):
    return np.float64(LAM) ** (-np.arange(T, dtype=np.float64))


def build_nc(T, Bc):
    """Build the Bass module for T steps, per-core batch Bc."""
    f32, f32r = dt.float32, dt.float32r
    nc = bacc.Bacc("TRN2", target_bir_lowering=False)

    xT_d = nc.dram_tensor("xT", [T, D_IN, Bc], f32r, kind="ExternalInput")
    w1hi_d = nc.dram_tensor("w1hi", [D_IN, HT], f32r, kind="ExternalInput")
    w1lo_d = nc.dram_tensor("w1lo", [D_IN, HT], f32r, kind="ExternalInput")
    w2hi_d = nc.dram_tensor("w2hi", [128, NCH * NOUT], f32r, kind="ExternalInput")
    w2lo_d = nc.dram_tensor("w2lo", [128, NCH * NOUT], f32r, kind="ExternalInput")
    negI_d = nc.dram_tensor("negI", [128, 128], f32r, kind="ExternalInput")
    negI21_d = nc.dram_tensor("negI21", [NOUT, NOUT], f32r, kind="ExternalInput")
    g_d = nc.dram_tensor("gmat", [NOUT, 2], f32r, kind="ExternalInput")
    th1_d = nc.dram_tensor("th1", [128, NCH * T], f32, kind="ExternalInput")
    th2_d = nc.dram_tensor("th2", [NOUT, T], f32, kind="ExternalInput")
    acc_d = nc.dram_tensor("acc", [2, Bc], f32, kind="ExternalOutput")
    d2c_d = nc.dram_tensor("d2c", [1, Bc], f32, kind="ExternalOutput")

    kap = [float(np.float32(k)) for k in kappas_f32(T)]

    FB = NCH * Bc                       # free width of packed layer-1 state
    REG = 2048 // 4                     # psum zero-region width in fp32 elems
    reg_w = min(FB, REG)
    n_reg = FB // reg_w
    ch_per_reg = reg_w // Bc

    with tile.TileContext(nc) as tc:
        with tc.tile_pool(name="consts", bufs=1) as consts, \
             tc.tile_pool(name="state", bufs=1) as state, \
             tc.tile_pool(name="xp", bufs=8) as xp, \
             tc.tile_pool(name="ps1p", bufs=2, space="PSUM") as ps1p, \
             tc.tile_pool(name="ps2p", bufs=2, space="PSUM") as ps2p, \
             tc.tile_pool(name="accp", bufs=1, space="PSUM") as accp:

            w1hi = consts.tile([D_IN, HT], f32r)
            w1lo = consts.tile([D_IN, HT], f32r)
            w2hi = consts.tile([128, NCH * NOUT], f32r)
            w2lo = consts.tile([128, NCH * NOUT], f32r)
            negI = consts.tile([128, 128], f32r)
            negI21 = consts.tile([NOUT, NOUT], f32r)
            gmat = consts.tile([NOUT, 2], f32r)
            th1 = consts.tile([128, NCH * T], f32)
            th2 = consts.tile([NOUT, T], f32)
            for tt, dd in ((w1hi, w1hi_d), (w1lo, w1lo_d), (w2hi, w2hi_d),
                           (w2lo, w2lo_d), (negI, negI_d), (negI21, negI21_d),
                           (gmat, g_d), (th1, th1_d), (th2, th2_d)):
                nc.gpsimd.dma_start(out=tt, in_=dd[:, :])

            Dp = state.tile([128, FB], f32)      # layer-1 D' (4 chunks packed)
            s1 = state.tile([128, FB], f32r)     # layer-1 spikes {0,1}
            D2 = state.tile([NOUT, Bc], f32)
            s2 = state.tile([NOUT, Bc], f32r)
            accsb = state.tile([2, Bc], f32)
            nc.vector.memset(Dp, 0.0)
            nc.vector.memset(D2, 0.0)
            # fp32r tiles can't be memset directly; produce rounded zeros via
            # an always-false compare (also satisfies the fp32r producer rule)
            nc.vector.tensor_scalar(out=s1, in0=Dp, scalar1=1e30, scalar2=None,
                                    op0=ALU.is_gt)
            nc.vector.tensor_scalar(out=s2, in0=D2, scalar1=1e30, scalar2=None,
                                    op0=ALU.is_gt)

            # Priming matmuls: each PE-consumed const gets one dummy matmul so
            # every later matmul carries at most one semaphore wait
            # (walrus rejects fp32r matmuls with >1 sync wait).
            pdum = ps2p.tile([NOUT, Bc], f32, tag="ps2")
            for cst, kk, mm in ((w1hi, D_IN, NOUT), (w1lo, D_IN, NOUT),
                                (w2hi, 128, NOUT), (w2lo, 128, NOUT),
                                (negI, 128, NOUT), (negI21, NOUT, NOUT),
                                (gmat, NOUT, 2)):
                nc.tensor.matmul(pdum[:mm, :2], cst[:kk, :mm], cst[:kk, :2],
                                 start=True, stop=True)

            acc_ps = accp.tile([2, Bc], f32)

            for t in range(T):
                x_t = xp.tile([D_IN, Bc], f32r)
                # alternate HWDGE rings (SP / ACT) for the x loads
                (nc.sync if t % 2 == 0 else nc.scalar).dma_start(
                    out=x_t, in_=xT_d[t, :, :])

                ps1 = ps1p.tile([128, FB], f32)
                for r in range(n_reg):
                    rsl = slice(r * reg_w, (r + 1) * reg_w)
                    # feed-forward matmuls first (off the recurrence chain)
                    for jj in range(ch_per_reg):
                        j = r * ch_per_reg + jj
                        sl = slice(j * Bc, (j + 1) * Bc)
                        wsl = slice(j * 128, (j + 1) * 128)
                        nc.tensor.matmul(ps1[:, sl], w1hi[:, wsl], x_t,
                                         start=(jj == 0), stop=False)
                        nc.tensor.matmul(ps1[:, sl], w1lo[:, wsl], x_t,
                                         start=False, stop=False)
                    # spike subtract for the whole region; last in group
                    nc.tensor.matmul(ps1[:, rsl], negI, s1[:, rsl],
                                     start=False, stop=True)
                # D' += kappa_t * psum  (one packed op, fewer DVE drains)
                nc.vector.scalar_tensor_tensor(
                    out=Dp, in0=ps1, scalar=kap[t], in1=Dp,
                    op0=ALU.mult, op1=ALU.add)
                # s = (D' > theta_t) per chunk
                for j in range(NCH):
                    sl = slice(j * Bc, (j + 1) * Bc)
                    nc.vector.tensor_scalar(
                        out=s1[:, sl], in0=Dp[:, sl],
                        scalar1=th1[:, j * T + t: j * T + t + 1],
                        scalar2=None, op0=ALU.is_gt)

                # layer 2
                ps2 = ps2p.tile([NOUT, Bc], f32)
                for j in range(NCH):
                    sl = slice(j * Bc, (j + 1) * Bc)
                    wsl = slice(j * NOUT, (j + 1) * NOUT)
                    nc.tensor.matmul(ps2, w2hi[:, wsl], s1[:, sl],
                                     start=(j == 0), stop=False)
                    nc.tensor.matmul(ps2, w2lo[:, wsl], s1[:, sl],
                                     start=False, stop=False)
                nc.tensor.matmul(ps2, negI21, s2, start=False, stop=True)

                nc.vector.scalar_tensor_tensor(
                    out=D2, in0=ps2, scalar=kap[t], in1=D2,
                    op0=ALU.mult, op1=ALU.add)
                nc.vector.tensor_scalar(
                    out=s2, in0=D2, scalar1=th2[:, t:t + 1], scalar2=None,
                    op0=ALU.is_gt)

                # spike counting: acc += G @ s2
                nc.tensor.matmul(acc_ps, gmat, s2,
                                 start=(t == 0), stop=(t == T - 1))

            nc.vector.tensor_copy(accsb, acc_ps)
            nc.sync.dma_start(out=acc_d[:, :], in_=accsb)
            nc.sync.dma_start(out=d2c_d[:, :], in_=D2[NOUT - 1:NOUT, :])

    nc.compile()
    return nc


def host_prep(T, Bc, W1a, b1a, W2a, b2a, W1c, b1c, W2c, b2c):
    """Precompute all constant arrays shared by every core."""
    W1 = np.vstack([W1a, W1c]).astype(np.float32)          # [512, 128]
    w1hi, w1lo = split_fp32r(np.ascontiguousarray(W1.T))   # [128, 512]

    W2blk = np.zeros((NOUT, HT), np.float32)
    W2blk[:2 * NPA, :H] = W2a
    W2blk[2 * NPA:, H:] = W2c
    W2T = np.ascontiguousarray(W2blk.T)                    # [512, 21]
    w2t = np.concatenate([W2T[j * 128:(j + 1) * 128, :] for j in range(NCH)],
                         axis=1)                           # [128, 84]
    w2hi, w2lo = split_fp32r(w2t)

    negI = np.ascontiguousarray(-np.eye(128, dtype=np.float32))
    negI21 = np.ascontiguousarray(-np.eye(NOUT, dtype=np.float32))
    G = np.zeros((NOUT, 2), np.float32)
    G[:NPA, 0] = 1.0
    G[NPA:2 * NPA, 1] = 1.0

    kap32 = np.float32(kappas_f32(T))
    S = np.cumsum(kap32.astype(np.float64))                # S_t

    b1 = np.concatenate([b1a, b1c]).astype(np.float32)     # [512]
    b2 = np.concatenate([b2a, b2c]).astype(np.float32)     # [21]
    beta1 = b1.astype(np.float64) - 0.05
    beta2 = b2.astype(np.float64) - 0.05
    # d = mem - 1 starts at -1: decayed initial condition adds +LAM
    th1_full = (LAM - beta1[:, None] * S[None, :]).astype(np.float32)  # [512, T]
    th1 = np.ascontiguousarray(
        th1_full.reshape(NCH, 128, T).transpose(1, 0, 2).reshape(128, NCH * T))
    th2 = (LAM - beta2[:, None] * S[None, :]).astype(np.float32)       # [21, T]

    consts = dict(w1hi=w1hi, w1lo=w1lo, w2hi=w2hi, w2lo=w2lo, negI=negI,
                  negI21=negI21, gmat=G, th1=th1,
                  th2=np.ascontiguousarray(th2))
    return consts, S


def host_post(acc_list, d2c_list, S, T, b2c):
    """Per-core [2,Bc]/[1,Bc] lists -> (policy [B,2], m_c2 [B,1])."""
    acc = np.concatenate(acc_list, axis=1)                 # [2, B]
    d2c = np.concatenate(d2c_list, axis=1)[0]              # [B]
    av = acc.T.astype(np.float32)                          # [B, 2] action values
    m = av.max(axis=1, keepdims=True)
    e = np.exp(av - m, dtype=np.float32)
    policy = (e / e.sum(axis=1, keepdims=True)).astype(np.float32)

    lam99 = np.float64(LAM) ** (T - 1)
    beta2c = np.float64(b2c[0]) - 0.05
    m_c2 = (lam99 * (d2c.astype(np.float64) + beta2c * S[T - 1] - LAM) + 1.0)
    return policy, m_c2.astype(np.float32)[:, None]


def run_full(spikes, weights, T=None, n_cores=N_CORES, **spmd_kwargs):
    """spikes [T,B,D] fp32 binary; weights dict W1a..b2c. Returns
    (policy [B,2], m_c2 [B,1], BassKernelResults)."""
    T = T if T is not None else spikes.shape[0]
    B = spikes.shape[1]
    Bc = B // n_cores
    consts, S = host_prep(T, Bc, **weights)
    nc = build_nc(T, Bc)
    xT = np.ascontiguousarray(spikes.transpose(0, 2, 1))   # [T, 128, B]
    in_maps = []
    for c in range(n_cores):
        m = dict(consts)
        m["xT"] = np.ascontiguousarray(xT[:, :, c * Bc:(c + 1) * Bc])
        in_maps.append(m)
    res = run_bass_kernel_spmd(nc, in_maps, core_ids=list(range(n_cores)),
                               **spmd_kwargs)
    acc_list = [res.results[c]["acc"] for c in range(n_cores)]
    d2c_list = [res.results[c]["d2c"] for c in range(n_cores)]
    policy, m_c2 = host_post(acc_list, d2c_list, S, T, weights["b2c"])
    return policy, m_c2, res


# ----------------------------------------------------------------------------
# Harness entry point: kernel(**inputs) -> (policy [B,2], m_c2 [B,1])
# ----------------------------------------------------------------------------
_NC_CACHE = {}
LAST_EXEC_NS = None


def kernel(spikes, W1a, b1a, W2a, b2a, W1c, b1c, W2c, b2c):
    global LAST_EXEC_NS
    spikes = np.ascontiguousarray(np.asarray(spikes, dtype=np.float32))
    weights = dict(W1a=np.asarray(W1a), b1a=np.asarray(b1a),
                   W2a=np.asarray(W2a), b2a=np.asarray(b2a),
                   W1c=np.asarray(W1c), b1c=np.asarray(b1c),
                   W2c=np.asarray(W2c), b2c=np.asarray(b2c))
    T, B, _ = spikes.shape
    Bc = B // N_CORES
    key = (T, Bc)
    if key not in _NC_CACHE:
        _NC_CACHE[key] = build_nc(T, Bc)
    nc = _NC_CACHE[key]

    consts, S = host_prep(T, Bc, **weights)
    xT = np.ascontiguousarray(spikes.transpose(0, 2, 1))   # [T, 128, B]
    in_maps = []
    for c in range(N_CORES):
        m = dict(consts)
        m["xT"] = np.ascontiguousarray(xT[:, :, c * Bc:(c + 1) * Bc])
        in_maps.append(m)
    res = run_bass_kernel_spmd(nc, in_maps, core_ids=list(range(N_CORES)))
    LAST_EXEC_NS = getattr(res, "exec_time_ns", None)
    acc_list = [res.results[c]["acc"] for c in range(N_CORES)]
    d2c_list = [res.results[c]["d2c"] for c in range(N_CORES)]
    policy, m_c2 = host_post(acc_list, d2c_list, S, T, weights["b2c"])
    return policy, m_c2


# revision 19
# speedup vs baseline: 1.2051x; 1.0152x over previous
"""SNN ActorCritic TRN2 kernel.

Math (per layer, snntorch Leaky, reset_mechanism='subtract', thresh=1):
    d_t = 0.95*d_{t-1} + (cur_t - 0.05) - s_{t-1},  s_t = (d_t > 0),  d = mem - 1

Rescaled state (kills the 0.95 multiply):  D_t = d_t / 0.95^t
    D_t = D_{t-1} + kappa_t * (g_t + beta - s_{t-1}),  kappa_t = 0.95^-t
where g_t = x_t @ W.T (matmul, no bias), beta = b - 0.05.
Kernel accumulates D'_t = sum kappa_tau*(g_tau - s_{tau-1}); the
deterministic beta part is folded into per-step thresholds:
    s_t = (D'_t > theta_t),  theta_t = LAM - beta*S_t  (LAM from d_{-1} = -1)

Per step on device:
    psum1[h,b] = sum_j W1hi/lo @ x_t  (2-term fp32r split, exact for binary x)
                 + (-I) @ s_{t-1}     (spike subtract via PE)
    D' += kappa_t * psum1             (DVE scalar_tensor_tensor, psum src)
    s   = (D' > theta_t)              (DVE tensor_scalar per chunk, fp32r out)
    psum2 = W2hi/lo @ s + (-I21) @ s2
    D2' += kappa_t * psum2;  s2 = (D2' > theta2_t)
    acc_psum += G @ s2                (spike counts, accumulated all T steps)

Outputs per core: acc [2,Bc] (policy logits = spike counts), D2'[20].
Host: softmax + affine reconstruction of m_c2.
"""
import numpy as np
import concourse.bass as bass
from concourse import bacc
import concourse.mybir as mybir
import concourse.tile as tile
from concourse.bass_utils import run_bass_kernel_spmd

dt = mybir.dt
ALU = mybir.AluOpType
AF = mybir.ActivationFunctionType

LAM = 0.95
D_IN = 128
H = 256
HT = 512          # actor H + critic H
NCH = 4           # HT / 128 partition chunks
NPA = 10
NOUT = 21         # 20 actor + 1 critic
N_CORES = 8


def split_fp32r(a):
    """Split fp32 array into hi+lo, both valid fp32r (low 12 mantissa bits 0),
    with hi+lo == a exactly."""
    a = np.ascontiguousarray(a, dtype=np.float32)
    hi = (a.view(np.uint32) & np.uint32(0xFFFFF000)).view(np.float32)
    lo = (a - hi).astype(np.float32)
    assert (lo.view(np.uint32) & np.uint32(0xFFF) == 0).all(), "lo not fp32r-valid"
    assert ((hi + lo) == a).all(), "split not exact"
    return hi, lo


def kappas_f32(T):
    return np.float64(LAM) ** (-np.arange(T, dtype=np.float64))


def build_nc(T, Bc):
    """Build the Bass module for T steps, per-core batch Bc."""
    f32, f32r = dt.float32, dt.float32r
    nc = bacc.Bacc("TRN2", target_bir_lowering=False)

    xT_d = nc.dram_tensor("xT", [T, D_IN, Bc], f32r, kind="ExternalInput")
    w1hi_d = nc.dram_tensor("w1hi", [D_IN, HT], f32r, kind="ExternalInput")
    w1lo_d = nc.dram_tensor("w1lo", [D_IN, HT], f32r, kind="ExternalInput")
    w2hi_d = nc.dram_tensor("w2hi", [128, NCH * NOUT], f32r, kind="ExternalInput")
    w2lo_d = nc.dram_tensor("w2lo", [128, NCH * NOUT], f32r, kind="ExternalInput")
    negI_d = nc.dram_tensor("negI", [128, 128], f32r, kind="ExternalInput")
    negI21_d = nc.dram_tensor("negI21", [NOUT, NOUT], f32r, kind="ExternalInput")
    g_d = nc.dram_tensor("gmat", [NOUT, 2], f32r, kind="ExternalInput")
    th1_d = nc.dram_tensor("th1", [128, NCH * T], f32, kind="ExternalInput")
    th2_d = nc.dram_tensor("th2", [NOUT, T], f32, kind="ExternalInput")
    acc_d = nc.dram_tensor("acc", [2, Bc], f32, kind="ExternalOutput")
    d2c_d = nc.dram_tensor("d2c", [1, Bc], f32, kind="ExternalOutput")

    kap = [float(np.float32(k)) for k in kappas_f32(T)]

    FB = NCH * Bc                       # free width of packed layer-1 state
    REG = 2048 // 4                     # psum zero-region width in fp32 elems
    reg_w = min(FB, REG)
    n_reg = FB // reg_w
    ch_per_reg = reg_w // Bc

    with tile.TileContext(nc) as tc:
        with tc.tile_pool(name="consts", bufs=1) as consts, \
             tc.tile_pool(name="state", bufs=1) as state, \
             tc.tile_pool(name="xp", bufs=8) as xp, \
             tc.tile_pool(name="ps1p", bufs=2, space="PSUM") as ps1p, \
             tc.tile_pool(name="ps2p", bufs=2, space="PSUM") as ps2p, \
             tc.tile_pool(name="accp", bufs=1, space="PSUM") as accp:

            w1hi = consts.tile([D_IN, HT], f32r)
            w1lo = consts.tile([D_IN, HT], f32r)
            w2hi = consts.tile([128, NCH * NOUT], f32r)
            w2lo = consts.tile([128, NCH * NOUT], f32r)
            negI = consts.tile([128, 128], f32r)
            negI21 = consts.tile([NOUT, NOUT], f32r)
            gmat = consts.tile([NOUT, 2], f32r)
            th1 = consts.tile([128, NCH * T], f32)
            th2 = consts.tile([NOUT, T], f32)
            for tt, dd in ((w1hi, w1hi_d), (w1lo, w1lo_d), (w2hi, w2hi_d),
                           (w2lo, w2lo_d), (negI, negI_d), (negI21, negI21_d),
                           (gmat, g_d), (th1, th1_d), (th2, th2_d)):
                nc.gpsimd.dma_start(out=tt, in_=dd[:, :])

            Dp = state.tile([128, FB], f32)      # layer-1 D' (4 chunks packed)
            s1 = state.tile([128, FB], f32r)     # layer-1 spikes {0,1}
            D2 = state.tile([NOUT, Bc], f32)
            s2 = state.tile([NOUT, Bc], f32r)
            accsb = state.tile([2, Bc], f32)
            nc.vector.memset(Dp, 0.0)
            nc.vector.memset(D2, 0.0)
            # fp32r tiles can't be memset directly; produce rounded zeros via
            # an always-false compare (also satisfies the fp32r producer rule)
            nc.vector.tensor_scalar(out=s1, in0=Dp, scalar1=1e30, scalar2=None,
                                    op0=ALU.is_gt)
            nc.vector.tensor_scalar(out=s2, in0=D2, scalar1=1e30, scalar2=None,
                                    op0=ALU.is_gt)

            # Priming matmuls: each PE-consumed const gets one dummy matmul so
            # every later matmul carries at most one semaphore wait
            # (walrus rejects fp32r matmuls with >1 sync wait).
            pdum = ps2p.tile([NOUT, Bc], f32, tag="ps2")
            for cst, kk, mm in ((w1hi, D_IN, NOUT), (w1lo, D_IN, NOUT),
                                (w2hi, 128, NOUT), (w2lo, 128, NOUT),
                                (negI, 128, NOUT), (negI21, NOUT, NOUT),
                                (gmat, NOUT, 2)):
                nc.tensor.matmul(pdum[:mm, :2], cst[:kk, :mm], cst[:kk, :2],
                                 start=True, stop=True)

            acc_ps = accp.tile([2, Bc], f32)

            for t in range(T):
                x_t = xp.tile([D_IN, Bc], f32r)
                # alternate HWDGE rings (SP / ACT) for the x loads
                (nc.sync if t % 2 == 0 else nc.scalar).dma_start(
                    out=x_t, in_=xT_d[t, :, :])

                ps1 = ps1p.tile([128, FB], f32)
                for r in range(n_reg):
                    rsl = slice(r * reg_w, (r + 1) * reg_w)
                    # feed-forward matmuls first (off the recurrence chain)
                    for jj in range(ch_per_reg):
                        j = r * ch_per_reg + jj
                        sl = slice(j * Bc, (j + 1) * Bc)
                        wsl = slice(j * 128, (j + 1) * 128)
                        nc.tensor.matmul(ps1[:, sl], w1hi[:, wsl], x_t,
                                         start=(jj == 0), stop=False)
                        nc.tensor.matmul(ps1[:, sl], w1lo[:, wsl], x_t,
                                         start=False, stop=False)
                    # spike subtract for the whole region; last in group
                    nc.tensor.matmul(ps1[:, rsl], negI, s1[:, rsl],
                                     start=False, stop=True)
                # D' += kappa_t * psum  (one packed op, fewer DVE drains)
                nc.vector.scalar_tensor_tensor(
                    out=Dp, in0=ps1, scalar=kap[t], in1=Dp,
                    op0=ALU.mult, op1=ALU.add)
                # s = (D' > theta_t) per chunk
                for j in range(NCH):
                    sl = slice(j * Bc, (j + 1) * Bc)
                    nc.vector.tensor_scalar(
                        out=s1[:, sl], in0=Dp[:, sl],
                        scalar1=th1[:, j * T + t: j * T + t + 1],
                        scalar2=None, op0=ALU.is_gt)

                # layer 2
                ps2 = ps2p.tile([NOUT, Bc], f32)
                for j in range(NCH):
                    sl = slice(j * Bc, (j + 1) * Bc)
                    wsl = slice(j * NOUT, (j + 1) * NOUT)
                    nc.tensor.matmul(ps2, w2hi[:, wsl], s1[:, sl],
                                     start=(j == 0), stop=False)
                    nc.tensor.matmul(ps2, w2lo[:, wsl], s1[:, sl],
                                     start=False, stop=False)
                nc.tensor.matmul(ps2, negI21, s2, start=False, stop=True)

                nc.vector.scalar_tensor_tensor(
                    out=D2, in0=ps2, scalar=kap[t], in1=D2,
                    op0=ALU.mult, op1=ALU.add)
                nc.vector.tensor_scalar(
                    out=s2, in0=D2, scalar1=th2[:, t:t + 1], scalar2=None,
                    op0=ALU.is_gt)

                # spike counting: acc += G @ s2
                nc.tensor.matmul(acc_ps, gmat, s2,
                                 start=(t == 0), stop=(t == T - 1))

            nc.vector.tensor_copy(accsb, acc_ps)
            nc.sync.dma_start(out=acc_d[:, :], in_=accsb)
            nc.sync.dma_start(out=d2c_d[:, :], in_=D2[NOUT - 1:NOUT, :])

    nc.compile()
    return nc


def host_prep(T, Bc, W1a, b1a, W2a, b2a, W1c, b1c, W2c, b2c):
    """Precompute all constant arrays shared by every core."""
    W1 = np.vstack([W1a, W1c]).astype(np.float32)          # [512, 128]
    w1hi, w1lo = split_fp32r(np.ascontiguousarray(W1.T))   # [128, 512]

    W2blk = np.zeros((NOUT, HT), np.float32)
    W2blk[:2 * NPA, :H] = W2a
    W2blk[2 * NPA:, H:] = W2c
    W2T = np.ascontiguousarray(W2blk.T)                    # [512, 21]
    w2t = np.concatenate([W2T[j * 128:(j + 1) * 128, :] for j in range(NCH)],
                         axis=1)                           # [128, 84]
    w2hi, w2lo = split_fp32r(w2t)

    negI = np.ascontiguousarray(-np.eye(128, dtype=np.float32))
    negI21 = np.ascontiguousarray(-np.eye(NOUT, dtype=np.float32))
    G = np.zeros((NOUT, 2), np.float32)
    G[:NPA, 0] = 1.0
    G[NPA:2 * NPA, 1] = 1.0

    kap32 = np.float32(kappas_f32(T))
    S = np.cumsum(kap32.astype(np.float64))                # S_t

    b1 = np.concatenate([b1a, b1c]).astype(np.float32)     # [512]
    b2 = np.concatenate([b2a, b2c]).astype(np.float32)     # [21]
    beta1 = b1.astype(np.float64) - 0.05
    beta2 = b2.astype(np.float64) - 0.05
    # d = mem - 1 starts at -1: decayed initial condition adds +LAM
    th1_full = (LAM - beta1[:, None] * S[None, :]).astype(np.float32)  # [512, T]
    th1 = np.ascontiguousarray(
        th1_full.reshape(NCH, 128, T).transpose(1, 0, 2).reshape(128, NCH * T))
    th2 = (LAM - beta2[:, None] * S[None, :]).astype(np.float32)       # [21, T]

    consts = dict(w1hi=w1hi, w1lo=w1lo, w2hi=w2hi, w2lo=w2lo, negI=negI,
                  negI21=negI21, gmat=G, th1=th1,
                  th2=np.ascontiguousarray(th2))
    return consts, S


def host_post(acc_list, d2c_list, S, T, b2c):
    """Per-core [2,Bc]/[1,Bc] lists -> (policy [B,2], m_c2 [B,1])."""
    acc = np.concatenate(acc_list, axis=1)                 # [2, B]
    d2c = np.concatenate(d2c_list, axis=1)[0]              # [B]
    av = acc.T.astype(np.float32)                          # [B, 2] action values
    m = av.max(axis=1, keepdims=True)
    e = np.exp(av - m, dtype=np.float32)
    policy = (e / e.sum(axis=1, keepdims=True)).astype(np.float32)

    lam99 = np.float64(LAM) ** (T - 1)
    beta2c = np.float64(b2c[0]) - 0.05
    m_c2 = (lam99 * (d2c.astype(np.float64) + beta2c * S[T - 1] - LAM) + 1.0)
    return policy, m_c2.astype(np.float32)[:, None]


def run_full(spikes, weights, T=None, n_cores=N_CORES, **spmd_kwargs):
    """spikes [T,B,D] fp32 binary; weights dict W1a..b2c. Returns
    (policy [B,2], m_c2 [B,1], BassKernelResults)."""
    T = T if T is not None else spikes.shape[0]
    B = spikes.shape[1]
    Bc = B // n_cores
    consts, S = host_prep(T, Bc, **weights)
    nc = build_nc(T, Bc)
    xT = np.ascontiguousarray(spikes.transpose(0, 2, 1))   # [T, 128, B]
    in_maps = []
    for c in range(n_cores):
        m = dict(consts)
        m["xT"] = np.ascontiguousarray(xT[:, :, c * Bc:(c + 1) * Bc])
        in_maps.append(m)
    res = run_bass_kernel_spmd(nc, in_maps, core_ids=list(range(n_cores)),
                               **spmd_kwargs)
    acc_list = [res.results[c]["acc"] for c in range(n_cores)]
    d2c_list = [res.results[c]["d2c"] for c in range(n_cores)]
    policy, m_c2 = host_post(acc_list, d2c_list, S, T, weights["b2c"])
    return policy, m_c2, res


# ----------------------------------------------------------------------------
# Harness entry point: kernel(**inputs) -> (policy [B,2], m_c2 [B,1])
# ----------------------------------------------------------------------------
_NC_CACHE = {}
LAST_EXEC_NS = None


def kernel(spikes, W1a, b1a, W2a, b2a, W1c, b1c, W2c, b2c):
    global LAST_EXEC_NS
    spikes = np.ascontiguousarray(np.asarray(spikes, dtype=np.float32))
    weights = dict(W1a=np.asarray(W1a), b1a=np.asarray(b1a),
                   W2a=np.asarray(W2a), b2a=np.asarray(b2a),
                   W1c=np.asarray(W1c), b1c=np.asarray(b1c),
                   W2c=np.asarray(W2c), b2c=np.asarray(b2c))
    T, B, _ = spikes.shape
    Bc = B // N_CORES
    key = (T, Bc)
    if key not in _NC_CACHE:
        _NC_CACHE[key] = build_nc(T, Bc)
    nc = _NC_CACHE[key]

    consts, S = host_prep(T, Bc, **weights)
    xT = np.ascontiguousarray(spikes.transpose(0, 2, 1))   # [T, 128, B]
    in_maps = []
    for c in range(N_CORES):
        m = dict(consts)
        m["xT"] = np.ascontiguousarray(xT[:, :, c * Bc:(c + 1) * Bc])
        in_maps.append(m)
    res = run_bass_kernel_spmd(nc, in_maps, core_ids=list(range(N_CORES)))
    LAST_EXEC_NS = getattr(res, "exec_time_ns", None)
    acc_list = [res.results[c]["acc"] for c in range(N_CORES)]
    d2c_list = [res.results[c]["d2c"] for c in range(N_CORES)]
    policy, m_c2 = host_post(acc_list, d2c_list, S, T, weights["b2c"])
    return policy, m_c2
